# revision 45
# baseline (speedup 1.0000x reference)
"""AdaptiveCfCCell fused kernel for 8 TRN2 NeuronCores (pure data parallel).

Reference computation (per sample row):
    xh = [x, h]                                  # [B, 4608]
    t  = gelu(LN(xh @ W1 + b1) * g1 + be1)       # [B, 512]
    p  = t @ W2 + b2                             # [B, 1536]
    tau = base_tau * 2*sigmoid(p[:, :512])
    A_row = 1 + 0.5*tanh(p[:, 512:1024])
    B_col = 1 + 0.5*tanh(p[:, 1024:1536])
    rec = (h @ base_A.T) * A_row
    inp = (x @ base_B) * B_col
    dhdt = -h/(tau+1e-8) + tanh(rec + inp)
    out = LN(h + 0.1*dhdt) * hn_g + hn_b

Sharding: batch dim split 8 ways; weights replicated. Activations are kept
batch-major (batch rows on SBUF partitions); x and h are pre-transposed on the
host so each matmul lhsT tile ([K=128, M=128]) DMAs contiguously. All matmuls
run in bf16 with f32 PSUM accumulation; normalization/elementwise math is f32.
Affine vectors that are exactly identity (b=0 / g=1) are skipped at build time.

Schedule: the first two batch tiles' MM1s are interleaved chunk-by-chunk so
the PE consumes each weight chunk twice as it arrives (the startup phase is
DMA-bound); after that, each iteration runs MM1(bt) while the previous tiles'
transpose+MM2+epilogue overlap it, with consecutive matmuls alternating PSUM
banks (216ns cadence vs 259ns for same-bank accumulation). Measured on
8xTRN2: ~193-198us whole-NEFF exec, rel err ~5e-4 vs the f32 reference.
"""

import sys

sys.path.insert(0, "/opt/trn_rl_repo")

import numpy as np
import ml_dtypes

import concourse.bass as bass
import concourse.tile as tile
import concourse.mybir as mybir
from concourse import bacc
from concourse.bass_utils import run_bass_kernel_spmd
from concourse.masks import make_identity

AF = mybir.ActivationFunctionType
ALU = mybir.AluOpType
F32 = mybir.dt.float32
BF16 = mybir.dt.bfloat16

D_IN, D_H, BATCH = 4096, 512, 8192
N_CORES = 8
B_LOC = BATCH // N_CORES          # 1024 rows per core
P = 128                           # partitions
NBT = B_LOC // P                  # 8 batch tiles per core
KX = D_IN // P                    # 32 K-chunks over x features
KH = D_H // P                     # 4  K-chunks over h features
N2 = 3 * D_H                      # 1536

_BF = ml_dtypes.bfloat16

_graph_cache: dict[tuple, object] = {}


def _bcast(ap, p=P):
    """Partition-broadcast a 1-D DRAM row vector AP to [p, n]."""
    return bass.AP(tensor=ap.tensor, offset=ap.offset, ap=[[0, p], *ap.ap])


def _build(flags: frozenset):
    """Build + compile the SPMD graph. `flags` names the non-identity affine
    vectors that must actually be applied."""
    use = lambda k: k in flags

    nc = bacc.Bacc("TRN2", target_bir_lowering=False)

    xT_e = nc.dram_tensor("xT", [NBT, P, KX, P], BF16, kind="ExternalInput")
    hT_e = nc.dram_tensor("hT", [NBT, P, KH, P], BF16, kind="ExternalInput")
    h_e = nc.dram_tensor("h", [B_LOC, D_H], F32, kind="ExternalInput")
    w1_e = nc.dram_tensor("w1", [P, KX + KH, D_H], BF16, kind="ExternalInput")
    wb_e = nc.dram_tensor("wb", [P, KX, D_H], BF16, kind="ExternalInput")
    wa_e = nc.dram_tensor("wa", [P, KH, D_H], BF16, kind="ExternalInput")
    w2_e = nc.dram_tensor("w2", [P, KH, N2], BF16, kind="ExternalInput")
    out_e = nc.dram_tensor("out", [B_LOC, D_H], F32, kind="ExternalOutput")

    vec_e = {}
    for name, n in [("b1", D_H), ("g1", D_H), ("be1", D_H), ("b2", N2),
                    ("btau", D_H), ("hng", D_H), ("hnb", D_H)]:
        if use(name):
            vec_e[name] = nc.dram_tensor(name, [n], F32, kind="ExternalInput")

    with tile.TileContext(nc) as tc:
        with (
            tc.tile_pool(name="weights", bufs=1) as wp,
            tc.tile_pool(name="stream", bufs=3) as sp,
            tc.tile_pool(name="work", bufs=3) as ep,
            tc.tile_pool(name="stats", bufs=3) as stp,
            tc.tile_pool(name="pst0", bufs=2, space="PSUM") as pst0,
            tc.tile_pool(name="psinp", bufs=1, space="PSUM") as psinp,
            tc.tile_pool(name="psps", bufs=5, space="PSUM") as psps,
        ):
            # ---- resident weights -------------------------------------
            ident = wp.tile([P, P], BF16, tag="ident")
            make_identity(nc, ident)
            eps_t = wp.tile([P, 1], F32, tag="eps")
            nc.vector.memset(eps_t, 1e-5)

            # The ~11MB weight stream is the startup bottleneck (the DMA
            # system sustains ~330GB/s): w1 slabs go on the gpsimd queue,
            # wb/wa/w2 on the scalar queue, activations on sync, all in the
            # order the (paired) MM1 consumes them.
            # weights live in per-slab tiles (one DMA each); w1sb/wbsb/...
            # below are per-chunk [P, 512] views into them
            W1S = [(0, 2), (2, 6), (6, 12), (12, 20), (20, 28), (28, 32),
                   (32, 36)]
            WBS = [(0, 2), (2, 6), (6, 12), (12, 20), (20, 28), (28, 32)]
            w1slab = [wp.tile([P, s1 - s0, D_H], BF16, tag=f"w1s_{k}",
                              name=f"w1s_{k}") for k, (s0, s1) in enumerate(W1S)]
            wbslab = [wp.tile([P, s1 - s0, D_H], BF16, tag=f"wbs_{k}",
                              name=f"wbs_{k}") for k, (s0, s1) in enumerate(WBS)]
            waslab = wp.tile([P, KH, D_H], BF16, tag="was", name="was")
            w2slab = wp.tile([P, KH, N2], BF16, tag="w2s", name="w2s")

            def _chunk_views(slabs, bounds):
                views = []
                for k, (s0, s1) in enumerate(bounds):
                    for j in range(s1 - s0):
                        views.append(slabs[k][:, j, :])
                return views

            w1sb = _chunk_views(w1slab, W1S)
            wbsb = _chunk_views(wbslab, WBS)
            wasb = [waslab[:, c, :] for c in range(KH)]
            w2sb = [w2slab[:, f, :] for f in range(KH)]

            # Weights spread over four DMA issue queues (gpsimd/scalar/
            # vector/sync), each in MM1(0) consumption order, so bt0/bt1 are
            # not gated on a single ~200GB/s queue. xt0 goes first on sync
            # in four 256KB slabs so MM#0 starts ~1us after the preamble.
            xt0 = sp.tile([P, KX, P], BF16, tag="xt")
            ht0 = sp.tile([P, KH, P], BF16, tag="ht")
            hn0 = sp.tile([P, D_H], F32, tag="hn")
            xt1 = sp.tile([P, KX, P], BF16, tag="xt", name="xt_1")
            ht1 = sp.tile([P, KH, P], BF16, tag="ht", name="ht_1")
            hn1 = sp.tile([P, D_H], F32, tag="hn", name="hn_1")
            # h-part inputs lead their queues (the paired MM1 starts with
            # the h-part); sync then carries the xt0/xt1 slab stream
            nc.sync.dma_start(out=ht0, in_=hT_e[0])
            nc.sync.dma_start(out=ht1, in_=hT_e[1])
            for s0, s1 in [(0, 8), (8, 20), (20, 32)]:
                nc.sync.dma_start(out=xt0[:, s0:s1, :],
                                  in_=xT_e[0][:, s0:s1, :])
                nc.sync.dma_start(out=xt1[:, s0:s1, :],
                                  in_=xT_e[1][:, s0:s1, :])

            # slab DMAs are fully contiguous (weights are stored
            # partition-major in DRAM); slabs are spread over the three DMA
            # queues interleaved by the order MM1(0) consumes them
            # first h-weight chunk on its own small DMA so MM#0 starts
            # ~2us earlier; slabs balanced across the three queues roughly
            # by byte count in consumption order
            nc.gpsimd.dma_start(out=w1slab[len(W1S) - 1][:, 0:1, :],
                                in_=w1_e[:, KX:KX + 1, :])
            nc.scalar.dma_start(out=waslab[:, 0:1, :], in_=wa_e[:, 0:1, :])
            nc.gpsimd.dma_start(out=w1slab[len(W1S) - 1][:, 1:KH, :],
                                in_=w1_e[:, KX + 1:KX + KH, :])
            nc.scalar.dma_start(out=waslab[:, 1:KH, :], in_=wa_e[:, 1:KH, :])
            for k, (s0, s1) in enumerate(W1S[:-1]):
                nc.gpsimd.dma_start(out=w1slab[k], in_=w1_e[:, s0:s1, :])
            for k, (s0, s1) in enumerate(WBS):
                nc.scalar.dma_start(out=wbslab[k], in_=wb_e[:, s0:s1, :])
            nc.gpsimd.dma_start(out=hn0, in_=h_e[0:P, :])
            nc.scalar.dma_start(out=w2slab, in_=w2_e[:, :, :])
            nc.scalar.dma_start(out=hn1, in_=h_e[P:2 * P, :])

            vecs = {}
            for name, n in [("b1", D_H), ("g1", D_H), ("be1", D_H), ("b2", N2),
                            ("btau", D_H), ("hng", D_H), ("hnb", D_H)]:
                if use(name):
                    t = wp.tile([P, n], F32, tag=f"vec_{name}")
                    nc.gpsimd.dma_start(out=t, in_=_bcast(vec_e[name][:]))
                    vecs[name] = t
            if use("btau"):
                t = wp.tile([P, D_H], F32, tag="vec_btau_inv")
                nc.vector.reciprocal(t, vecs["btau"])
                vecs["btau_inv"] = t

            # ---- main loop over batch tiles ---------------------------
            # Software-pipelined: iteration bt emits MM1(bt) (t0/inp/rec
            # matmuls interleaved so consecutive matmuls hit alternating
            # PSUM banks -> ~216ns cadence), then transpose+MM2+epilogue of
            # bt-1 (their PE work lands after MM1(bt), hiding the LN1/gelu
            # latency), then LN1(bt). ACT order per iteration is
            # [Exp,Tanh x3,Sqrt | Sqrt,Gelu] -> 4 table loads per tile.
            state = {}

            def emit_mm1_multi(bts, xts, hts, hns):
                """MM1 for several batch tiles interleaved chunk-by-chunk:
                each weight chunk is consumed len(bts) times on arrival, so
                the PE stays busy while the 11MB weight stream is still in
                flight (the first tiles are DMA-bound, not PE-bound). The
                h-part runs first (it needs only ~1.1MB of inputs) and its
                rec accumulators are evicted immediately to free banks."""
                n = len(bts)
                t0s, inps, recs = [], [], []
                for j, b in enumerate(bts):
                    pool = pst0 if j < 2 else psps
                    t0s.append(pool.tile([P, D_H], F32,
                                         tag="t0" if j < 2 else "ps",
                                         name=f"t0_{b}"))
                    pool = psinp if j == 0 else psps
                    inps.append(pool.tile([P, D_H], F32,
                                          tag="inp" if j == 0 else "ps",
                                          name=f"inp_{b}"))
                    recs.append(psps.tile([P, D_H], F32, tag="ps",
                                          name=f"rec_{b}"))
                for c in range(KH):
                    for j in range(n):
                        nc.tensor.matmul(t0s[j], lhsT=hts[j][:, c, :],
                                         rhs=w1sb[KX + c], start=(c == 0),
                                         stop=False)
                    for j in range(n):
                        nc.tensor.matmul(recs[j], lhsT=hts[j][:, c, :],
                                         rhs=wasb[c], start=(c == 0),
                                         stop=(c == KH - 1))
                rec_ss = []
                for j, b in enumerate(bts):
                    rec_s = ep.tile([P, D_H], F32, tag="rec_s",
                                    name=f"recs_{b}")
                    nc.vector.tensor_copy(out=rec_s, in_=recs[j])
                    rec_ss.append(rec_s)
                for c in range(KX):
                    for j in range(n):
                        nc.tensor.matmul(t0s[j], lhsT=xts[j][:, c, :],
                                         rhs=w1sb[c], start=False,
                                         stop=(c == KX - 1))
                    for j in range(n):
                        nc.tensor.matmul(inps[j], lhsT=xts[j][:, c, :],
                                         rhs=wbsb[c], start=(c == 0),
                                         stop=(c == KX - 1))
                for j, b in enumerate(bts):
                    inp_s = ep.tile([P, D_H], F32, tag="inp_s",
                                    name=f"inps_{b}")
                    nc.vector.tensor_copy(out=inp_s, in_=inps[j])
                    state[b] = [t0s[j], inp_s, rec_ss[j], hns[j], None]

            def emit_mm1(bt, xt, ht, hn):
                t0 = pst0.tile([P, D_H], F32, tag="t0", name=f"t0_{bt}")
                inp = psinp.tile([P, D_H], F32, tag="inp", name=f"inp_{bt}")
                rec = psps.tile([P, D_H], F32, tag="ps", name=f"rec_{bt}")
                seqA = [(t0, xt[:, c, :], w1sb[c], c == 0, False)
                        for c in range(KX)]
                seqA += [(t0, ht[:, c, :], w1sb[KX + c], False, c == KH - 1)
                         for c in range(KH)]
                seqB = [(inp, xt[:, c, :], wbsb[c], c == 0, c == KX - 1)
                        for c in range(KX)]
                seqB += [(rec, ht[:, c, :], wasb[c], c == 0, c == KH - 1)
                         for c in range(KH)]
                # A0 A1 (B0 A2) (B1 A3) ... : banks alternate and inp's
                # first matmul trails the previous tile's PSUM eviction.
                order = seqA[:2]
                for i in range(len(seqB)):
                    order.append(seqB[i])
                    if i + 2 < len(seqA):
                        order.append(seqA[i + 2])
                for out_ps, lhsT, rhs, st, sp_ in order:
                    nc.tensor.matmul(out_ps, lhsT=lhsT, rhs=rhs,
                                     start=st, stop=sp_)

                # evict inp/rec to SBUF right away so their banks free for
                # the next tile's MM1 (inp bufs=1, rec shares the ps pool)
                inp_s = ep.tile([P, D_H], F32, tag="inp_s", name=f"inps_{bt}")
                nc.vector.tensor_copy(out=inp_s, in_=inp)
                rec_s = ep.tile([P, D_H], F32, tag="rec_s", name=f"recs_{bt}")
                nc.vector.tensor_copy(out=rec_s, in_=rec)
                state[bt] = [t0, inp_s, rec_s, hn, None]

            def emit_ln1(bt):
                t0, inp_s, rec_s, hn, _ = state[bt]
                if use("b1"):
                    nc.vector.tensor_add(t0, t0, vecs["b1"])
                st1 = stp.tile([P, 6], F32, tag="st1", name=f"st1_{bt}")
                nc.vector.bn_stats(st1, t0)
                mv1 = stp.tile([P, 2], F32, tag="mv1", name=f"mv1_{bt}")
                nc.vector.bn_aggr(mv1, st1)
                sd1 = stp.tile([P, 1], F32, tag="sd1", name=f"sd1_{bt}")
                nc.scalar.activation(sd1, mv1[:, 1:2], AF.Sqrt, bias=eps_t)
                rs1 = stp.tile([P, 1], F32, tag="rs1", name=f"rs1_{bt}")
                nc.vector.reciprocal(rs1, sd1)

                t2b = sp.tile([P, D_H], BF16, tag="t2b", name=f"t2b_{bt}")
                if use("g1") or use("be1"):
                    t1 = ep.tile([P, D_H], F32, tag="t1", name=f"t1_{bt}")
                    nc.vector.tensor_scalar(t1, t0, mv1[:, 0:1], rs1,
                                            ALU.subtract, ALU.mult)
                    if use("g1"):
                        nc.vector.tensor_mul(t1, t1, vecs["g1"])
                    if use("be1"):
                        nc.vector.tensor_add(t1, t1, vecs["be1"])
                    nc.scalar.activation(t2b, t1, AF.Gelu)
                else:
                    nmr = stp.tile([P, 1], F32, tag="nmr", name=f"nmr_{bt}")
                    nc.vector.tensor_scalar(nmr, mv1[:, 0:1], rs1, -1.0,
                                            ALU.mult, ALU.mult)
                    nc.scalar.activation(t2b, t0, AF.Gelu, bias=nmr, scale=rs1)
                state[bt][4] = t2b

            heads = {}

            def emit_head(bt, alt_params=False):
                t0, inp_s, rec_s, hn, t2b = state.pop(bt)

                t2T = sp.tile([P, KH, P], BF16, tag="t2T", name=f"t2T_{bt}")
                for f in range(KH):
                    tp = psps.tile([P, P], BF16, tag="ps", name=f"tp_{bt}_{f}")
                    nc.tensor.transpose(tp, t2b[:, f * P:(f + 1) * P], ident)
                    nc.vector.tensor_copy(out=t2T[:, f, :], in_=tp)

                if alt_params:
                    # the MM1 pools are idle once the last MM1 has been
                    # consumed - reuse their banks so this tail's MM2 does
                    # not wait for the previous tail's ACT chain to release
                    # the shared pool's slots
                    taus = pst0.tile([P, D_H], F32, tag="t0",
                                     name=f"taus_{bt}")
                    As = pst0.tile([P, D_H], F32, tag="t0", name=f"As_{bt}")
                    Bs = psinp.tile([P, D_H], F32, tag="inp",
                                    name=f"Bs_{bt}")
                else:
                    taus = psps.tile([P, D_H], F32, tag="ps",
                                     name=f"taus_{bt}")
                    As = psps.tile([P, D_H], F32, tag="ps", name=f"As_{bt}")
                    Bs = psps.tile([P, D_H], F32, tag="ps", name=f"Bs_{bt}")
                for f in range(KH):
                    nc.tensor.matmul(taus, lhsT=t2T[:, f, :],
                                     rhs=w2sb[f][:, 0:D_H],
                                     start=(f == 0), stop=(f == KH - 1))
                    nc.tensor.matmul(As, lhsT=t2T[:, f, :],
                                     rhs=w2sb[f][:, D_H:2 * D_H],
                                     start=(f == 0), stop=(f == KH - 1))
                    nc.tensor.matmul(Bs, lhsT=t2T[:, f, :],
                                     rhs=w2sb[f][:, 2 * D_H:N2],
                                     start=(f == 0), stop=(f == KH - 1))
                if use("b2"):
                    nc.vector.tensor_add(taus, taus, vecs["b2"][:, 0:D_H])
                    nc.vector.tensor_add(As, As, vecs["b2"][:, D_H:2 * D_H])
                    nc.vector.tensor_add(Bs, Bs, vecs["b2"][:, 2 * D_H:N2])
                heads[bt] = (taus, As, Bs, inp_s, rec_s, hn)

            def emit_epi(bt):
                taus, As, Bs, inp_s, rec_s, hn = heads.pop(bt)
                sg = ep.tile([P, D_H], F32, tag="sg", name=f"sg_{bt}")
                tA = ep.tile([P, D_H], F32, tag="tA", name=f"tA_{bt}")
                tB = ep.tile([P, D_H], F32, tag="tB", name=f"tB_{bt}")
                st2 = stp.tile([P, 6], F32, tag="st2", name=f"st2_{bt}")

                # 0.1*h/(2*sigmoid(x)+1e-8) ~= 0.05*h*(1+exp(-x)): exact
                # sigmoid identity; avoids the slow DVE reciprocal
                nc.scalar.activation(sg, taus, AF.Exp, scale=-1.0)
                nc.vector.tensor_scalar(sg, sg, 1.0, 0.05, ALU.add, ALU.mult)
                if use("btau"):
                    nc.vector.tensor_mul(sg, sg, vecs["btau_inv"])
                nc.vector.tensor_mul(sg, hn, sg)
                nc.scalar.activation(tB, Bs, AF.Tanh)
                nc.vector.tensor_scalar(tB, tB, 0.5, 1.0, ALU.mult, ALU.add)
                nc.vector.tensor_mul(tB, inp_s, tB)        # inp * B_col
                nc.scalar.activation(tA, As, AF.Tanh)
                nc.vector.tensor_scalar(tA, tA, 0.5, 1.0, ALU.mult, ALU.add)
                nc.vector.tensor_mul(tA, rec_s, tA)        # rec * A_row
                nc.vector.tensor_add(tA, tA, tB)
                nc.scalar.activation(tA, tA, AF.Tanh)      # tanh(rec'+inp')
                nc.vector.tensor_scalar(tA, tA, 0.1, None, ALU.mult)
                nc.vector.tensor_sub(tA, tA, sg)           # 0.1*dhdt
                nc.vector.tensor_add(tA, hn, tA)           # h + 0.1*dhdt
                nc.vector.bn_stats(st2, tA)
                mv2 = stp.tile([P, 2], F32, tag="mv2", name=f"mv2_{bt}")
                nc.vector.bn_aggr(mv2, st2)
                sd2 = stp.tile([P, 1], F32, tag="sd2", name=f"sd2_{bt}")
                nc.scalar.activation(sd2, mv2[:, 1:2], AF.Sqrt, bias=eps_t)
                rs2 = stp.tile([P, 1], F32, tag="rs2", name=f"rs2_{bt}")
                nc.vector.reciprocal(rs2, sd2)

                y = ep.tile([P, D_H], F32, tag="y", name=f"y_{bt}")
                nc.vector.tensor_scalar(y, tA, mv2[:, 0:1], rs2,
                                        ALU.subtract, ALU.mult)
                if use("hng"):
                    nc.vector.tensor_mul(y, y, vecs["hng"])
                if use("hnb"):
                    nc.vector.tensor_add(y, y, vecs["hnb"])
                nc.sync.dma_start(out=out_e[bt * P:(bt + 1) * P, :], in_=y)

            def emit_tail(bt, alt_params=False):
                emit_head(bt, alt_params=alt_params)
                emit_epi(bt)

            def emit_mm1_p1(bt, xt, ht, hn):
                """First half of the last tile's MM1: only the t0
                accumulation, so gelu and both remaining heads can run
                while the inp/rec half still has PE work left."""
                t0 = pst0.tile([P, D_H], F32, tag="t0", name=f"t0_{bt}")
                inp = psinp.tile([P, D_H], F32, tag="inp", name=f"inp_{bt}")
                rec = psps.tile([P, D_H], F32, tag="ps", name=f"rec_{bt}")
                for c in range(KX):
                    nc.tensor.matmul(t0, lhsT=xt[:, c, :], rhs=w1sb[c],
                                     start=(c == 0), stop=False)
                for c in range(KH):
                    nc.tensor.matmul(t0, lhsT=ht[:, c, :], rhs=w1sb[KX + c],
                                     start=False, stop=(c == KH - 1))
                # epi reads inp/rec straight from PSUM for this tile
                state[bt] = [t0, inp, rec, hn, None]
                return inp, rec

            def emit_mm1_p2(bt, xt, ht, inp, rec):
                for c in range(KH):
                    nc.tensor.matmul(rec, lhsT=ht[:, c, :], rhs=wasb[c],
                                     start=(c == 0), stop=(c == KH - 1))
                for c in range(KX):
                    nc.tensor.matmul(inp, lhsT=xt[:, c, :], rhs=wbsb[c],
                                     start=(c == 0), stop=(c == KX - 1))

            emit_mm1_multi((0, 1), (xt0, xt1), (ht0, ht1), (hn0, hn1))
            emit_ln1(0)
            emit_ln1(1)
            for bt in range(2, NBT):
                xt = sp.tile([P, KX, P], BF16, tag="xt", name=f"xt_{bt}")
                nc.sync.dma_start(out=xt, in_=xT_e[bt])
                ht = sp.tile([P, KH, P], BF16, tag="ht", name=f"ht_{bt}")
                nc.sync.dma_start(out=ht, in_=hT_e[bt])
                hn = sp.tile([P, D_H], F32, tag="hn", name=f"hn_{bt}")
                nc.sync.dma_start(out=hn, in_=h_e[bt * P:(bt + 1) * P, :])
                if bt == NBT - 1:
                    # tail(5) moves ahead of MM1(7); MM1(7) itself is split:
                    # t0-half first, then gelu(7) + heads(6,7), then the
                    # inp/rec half, so both remaining epilogues overlap
                    # ~8us of PE work instead of trailing the last matmul
                    emit_head(bt - 2)
                    emit_epi(bt - 2)
                    inp7, rec7 = emit_mm1_p1(bt, xt, ht, hn)
                    emit_ln1(bt)
                    emit_head(bt - 1, alt_params=True)
                    emit_head(bt)
                    emit_mm1_p2(bt, xt, ht, inp7, rec7)
                    emit_epi(bt - 1)
                    emit_epi(bt)
                else:
                    emit_mm1(bt, xt, ht, hn)
                    emit_tail(bt - 2)
                    emit_ln1(bt)

    nc.compile()
    return nc


def _prep(x, h, W1, b1, ln1_g, ln1_b, W2, b2, base_tau, base_A, base_B,
          hn_g, hn_b):
    """Host-side shard prep. Returns (flags, in_maps)."""
    flags = set()
    checks = [("b1", b1, 0.0), ("g1", ln1_g, 1.0), ("be1", ln1_b, 0.0),
              ("b2", b2, 0.0), ("btau", base_tau, 1.0), ("hng", hn_g, 1.0),
              ("hnb", hn_b, 0.0)]
    vals = {"b1": b1, "g1": ln1_g, "be1": ln1_b, "b2": b2, "btau": base_tau,
            "hng": hn_g, "hnb": hn_b}
    for name, v, ident_val in checks:
        if not np.all(np.asarray(v) == ident_val):
            flags.add(name)

    w1 = np.ascontiguousarray(
        W1.reshape(KX + KH, P, D_H).transpose(1, 0, 2)).astype(_BF)
    wb = np.ascontiguousarray(
        base_B.reshape(KX, P, D_H).transpose(1, 0, 2)).astype(_BF)
    wa = np.ascontiguousarray(
        base_A.T.reshape(KH, P, D_H).transpose(1, 0, 2)).astype(_BF)
    w2 = np.ascontiguousarray(
        W2.reshape(KH, P, N2).transpose(1, 0, 2)).astype(_BF)

    in_maps = []
    for i in range(N_CORES):
        xl = x[i * B_LOC:(i + 1) * B_LOC]
        hl = h[i * B_LOC:(i + 1) * B_LOC]
        # xT[bt, k, c, b] = x[bt*128+b, c*128+k]
        xT = np.ascontiguousarray(
            xl.reshape(NBT, P, KX, P).transpose(0, 3, 2, 1)).astype(_BF)
        hT = np.ascontiguousarray(
            hl.reshape(NBT, P, KH, P).transpose(0, 3, 2, 1)).astype(_BF)
        m = {"xT": xT, "hT": hT, "h": np.ascontiguousarray(hl, np.float32),
             "w1": w1, "wb": wb, "wa": wa, "w2": w2}
        for name in flags:
            m[name] = np.ascontiguousarray(vals[name], np.float32)
        in_maps.append(m)
    return frozenset(flags), in_maps


def run(inputs: dict, trace: bool = False):
    """Compile (cached), execute on cores 0-7, gather. Returns (out, res)."""
    flags, in_maps = _prep(**inputs)
    nc = _graph_cache.get(flags)
    if nc is None:
        nc = _build(flags)
        _graph_cache[flags] = nc
    res = run_bass_kernel_spmd(nc, in_maps, core_ids=list(range(N_CORES)),
                               trace=trace)
    out = np.concatenate([res.results[i]["out"] for i in range(N_CORES)],
                         axis=0)
    return out, res


def kernel(**inputs) -> np.ndarray:
    out, _ = run(inputs, trace=False)
    return out


# revision 46
# speedup vs baseline: 1.0086x; 1.0086x over previous
"""AdaptiveCfCCell fused kernel for 8 TRN2 NeuronCores (pure data parallel).

Reference computation (per sample row):
    xh = [x, h]                                  # [B, 4608]
    t  = gelu(LN(xh @ W1 + b1) * g1 + be1)       # [B, 512]
    p  = t @ W2 + b2                             # [B, 1536]
    tau = base_tau * 2*sigmoid(p[:, :512])
    A_row = 1 + 0.5*tanh(p[:, 512:1024])
    B_col = 1 + 0.5*tanh(p[:, 1024:1536])
    rec = (h @ base_A.T) * A_row
    inp = (x @ base_B) * B_col
    dhdt = -h/(tau+1e-8) + tanh(rec + inp)
    out = LN(h + 0.1*dhdt) * hn_g + hn_b

Sharding: batch dim split 8 ways; weights replicated. Activations are kept
batch-major (batch rows on SBUF partitions); x and h are pre-transposed on the
host so each matmul lhsT tile ([K=128, M=128]) DMAs contiguously. All matmuls
run in bf16 with f32 PSUM accumulation; normalization/elementwise math is f32.
Affine vectors that are exactly identity (b=0 / g=1) are skipped at build time.

Schedule: the first two batch tiles' MM1s are interleaved chunk-by-chunk so
the PE consumes each weight chunk twice as it arrives (the startup phase is
DMA-bound); after that, each iteration runs MM1(bt) while the previous tiles'
transpose+MM2+epilogue overlap it, with consecutive matmuls alternating PSUM
banks (216ns cadence vs 259ns for same-bank accumulation). Measured on
8xTRN2: ~193-198us whole-NEFF exec, rel err ~5e-4 vs the f32 reference.
"""

import sys

sys.path.insert(0, "/opt/trn_rl_repo")

import numpy as np
import ml_dtypes

import concourse.bass as bass
import concourse.tile as tile
import concourse.mybir as mybir
from concourse import bacc
from concourse.bass_utils import run_bass_kernel_spmd
from concourse.masks import make_identity

AF = mybir.ActivationFunctionType
ALU = mybir.AluOpType
F32 = mybir.dt.float32
BF16 = mybir.dt.bfloat16

D_IN, D_H, BATCH = 4096, 512, 8192
N_CORES = 8
B_LOC = BATCH // N_CORES          # 1024 rows per core
P = 128                           # partitions
NBT = B_LOC // P                  # 8 batch tiles per core
KX = D_IN // P                    # 32 K-chunks over x features
KH = D_H // P                     # 4  K-chunks over h features
N2 = 3 * D_H                      # 1536

_BF = ml_dtypes.bfloat16

_graph_cache: dict[tuple, object] = {}


def _bcast(ap, p=P):
    """Partition-broadcast a 1-D DRAM row vector AP to [p, n]."""
    return bass.AP(tensor=ap.tensor, offset=ap.offset, ap=[[0, p], *ap.ap])


def _build(flags: frozenset):
    """Build + compile the SPMD graph. `flags` names the non-identity affine
    vectors that must actually be applied."""
    use = lambda k: k in flags

    nc = bacc.Bacc("TRN2", target_bir_lowering=False)

    xT_e = nc.dram_tensor("xT", [NBT, P, KX, P], BF16, kind="ExternalInput")
    hT_e = nc.dram_tensor("hT", [NBT, P, KH, P], BF16, kind="ExternalInput")
    h_e = nc.dram_tensor("h", [B_LOC, D_H], F32, kind="ExternalInput")
    w1_e = nc.dram_tensor("w1", [P, KX + KH, D_H], BF16, kind="ExternalInput")
    wb_e = nc.dram_tensor("wb", [P, KX, D_H], BF16, kind="ExternalInput")
    wa_e = nc.dram_tensor("wa", [P, KH, D_H], BF16, kind="ExternalInput")
    w2_e = nc.dram_tensor("w2", [P, KH, N2], BF16, kind="ExternalInput")
    out_e = nc.dram_tensor("out", [B_LOC, D_H], F32, kind="ExternalOutput")

    vec_e = {}
    for name, n in [("b1", D_H), ("g1", D_H), ("be1", D_H), ("b2", N2),
                    ("btau", D_H), ("hng", D_H), ("hnb", D_H)]:
        if use(name):
            vec_e[name] = nc.dram_tensor(name, [n], F32, kind="ExternalInput")

    with tile.TileContext(nc) as tc:
        with (
            tc.tile_pool(name="weights", bufs=1) as wp,
            tc.tile_pool(name="stream", bufs=3) as sp,
            tc.tile_pool(name="work", bufs=3) as ep,
            tc.tile_pool(name="stats", bufs=3) as stp,
            tc.tile_pool(name="pst0", bufs=2, space="PSUM") as pst0,
            tc.tile_pool(name="psinp", bufs=1, space="PSUM") as psinp,
            tc.tile_pool(name="psps", bufs=5, space="PSUM") as psps,
        ):
            # ---- resident weights -------------------------------------
            ident = wp.tile([P, P], BF16, tag="ident")
            make_identity(nc, ident)
            eps_t = wp.tile([P, 1], F32, tag="eps")
            nc.vector.memset(eps_t, 1e-5)

            # The ~11MB weight stream is the startup bottleneck (the DMA
            # system sustains ~330GB/s): w1 slabs go on the gpsimd queue,
            # wb/wa/w2 on the scalar queue, activations on sync, all in the
            # order the (paired) MM1 consumes them.
            # weights live in per-slab tiles (one DMA each); w1sb/wbsb/...
            # below are per-chunk [P, 512] views into them
            W1S = [(0, 2), (2, 6), (6, 12), (12, 20), (20, 28), (28, 32),
                   (32, 36)]
            WBS = [(0, 2), (2, 6), (6, 12), (12, 20), (20, 28), (28, 32)]
            w1slab = [wp.tile([P, s1 - s0, D_H], BF16, tag=f"w1s_{k}",
                              name=f"w1s_{k}") for k, (s0, s1) in enumerate(W1S)]
            wbslab = [wp.tile([P, s1 - s0, D_H], BF16, tag=f"wbs_{k}",
                              name=f"wbs_{k}") for k, (s0, s1) in enumerate(WBS)]
            waslab = wp.tile([P, KH, D_H], BF16, tag="was", name="was")
            w2slab = wp.tile([P, KH, N2], BF16, tag="w2s", name="w2s")

            def _chunk_views(slabs, bounds):
                views = []
                for k, (s0, s1) in enumerate(bounds):
                    for j in range(s1 - s0):
                        views.append(slabs[k][:, j, :])
                return views

            w1sb = _chunk_views(w1slab, W1S)
            wbsb = _chunk_views(wbslab, WBS)
            wasb = [waslab[:, c, :] for c in range(KH)]
            w2sb = [w2slab[:, f, :] for f in range(KH)]

            # Weights spread over four DMA issue queues (gpsimd/scalar/
            # vector/sync), each in MM1(0) consumption order, so bt0/bt1 are
            # not gated on a single ~200GB/s queue. xt0 goes first on sync
            # in four 256KB slabs so MM#0 starts ~1us after the preamble.
            xt0 = sp.tile([P, KX, P], BF16, tag="xt")
            ht0 = sp.tile([P, KH, P], BF16, tag="ht")
            hn0 = sp.tile([P, D_H], F32, tag="hn")
            xt1 = sp.tile([P, KX, P], BF16, tag="xt", name="xt_1")
            ht1 = sp.tile([P, KH, P], BF16, tag="ht", name="ht_1")
            hn1 = sp.tile([P, D_H], F32, tag="hn", name="hn_1")
            # h-part inputs lead their queues (the paired MM1 starts with
            # the h-part); sync then carries the xt0/xt1 slab stream
            nc.sync.dma_start(out=ht0, in_=hT_e[0])
            nc.sync.dma_start(out=ht1, in_=hT_e[1])
            for s0, s1 in [(0, 8), (8, 20), (20, 32)]:
                nc.sync.dma_start(out=xt0[:, s0:s1, :],
                                  in_=xT_e[0][:, s0:s1, :])
                nc.sync.dma_start(out=xt1[:, s0:s1, :],
                                  in_=xT_e[1][:, s0:s1, :])

            # slab DMAs are fully contiguous (weights are stored
            # partition-major in DRAM); slabs are spread over the three DMA
            # queues interleaved by the order MM1(0) consumes them
            # first h-weight chunk on its own small DMA so MM#0 starts
            # ~2us earlier; slabs balanced across the three queues roughly
            # by byte count in consumption order
            nc.gpsimd.dma_start(out=w1slab[len(W1S) - 1][:, 0:1, :],
                                in_=w1_e[:, KX:KX + 1, :])
            nc.scalar.dma_start(out=waslab[:, 0:1, :], in_=wa_e[:, 0:1, :])
            nc.gpsimd.dma_start(out=w1slab[len(W1S) - 1][:, 1:KH, :],
                                in_=w1_e[:, KX + 1:KX + KH, :])
            nc.scalar.dma_start(out=waslab[:, 1:KH, :], in_=wa_e[:, 1:KH, :])
            for k, (s0, s1) in enumerate(W1S[:-1]):
                nc.gpsimd.dma_start(out=w1slab[k], in_=w1_e[:, s0:s1, :])
            for k, (s0, s1) in enumerate(WBS):
                nc.scalar.dma_start(out=wbslab[k], in_=wb_e[:, s0:s1, :])
            nc.gpsimd.dma_start(out=hn0, in_=h_e[0:P, :])
            nc.scalar.dma_start(out=w2slab, in_=w2_e[:, :, :])
            nc.scalar.dma_start(out=hn1, in_=h_e[P:2 * P, :])

            vecs = {}
            for name, n in [("b1", D_H), ("g1", D_H), ("be1", D_H), ("b2", N2),
                            ("btau", D_H), ("hng", D_H), ("hnb", D_H)]:
                if use(name):
                    t = wp.tile([P, n], F32, tag=f"vec_{name}")
                    nc.gpsimd.dma_start(out=t, in_=_bcast(vec_e[name][:]))
                    vecs[name] = t
            if use("btau"):
                t = wp.tile([P, D_H], F32, tag="vec_btau_inv")
                nc.vector.reciprocal(t, vecs["btau"])
                vecs["btau_inv"] = t

            # ---- main loop over batch tiles ---------------------------
            # Software-pipelined: iteration bt emits MM1(bt) (t0/inp/rec
            # matmuls interleaved so consecutive matmuls hit alternating
            # PSUM banks -> ~216ns cadence), then transpose+MM2+epilogue of
            # bt-1 (their PE work lands after MM1(bt), hiding the LN1/gelu
            # latency), then LN1(bt). ACT order per iteration is
            # [Exp,Tanh x3,Sqrt | Sqrt,Gelu] -> 4 table loads per tile.
            state = {}

            def emit_mm1_multi(bts, xts, hts, hns):
                """MM1 for several batch tiles interleaved chunk-by-chunk:
                each weight chunk is consumed len(bts) times on arrival, so
                the PE stays busy while the 11MB weight stream is still in
                flight (the first tiles are DMA-bound, not PE-bound). The
                h-part runs first (it needs only ~1.1MB of inputs) and its
                rec accumulators are evicted immediately to free banks."""
                n = len(bts)
                t0s, inps, recs = [], [], []
                for j, b in enumerate(bts):
                    pool = pst0 if j < 2 else psps
                    t0s.append(pool.tile([P, D_H], F32,
                                         tag="t0" if j < 2 else "ps",
                                         name=f"t0_{b}"))
                    pool = psinp if j == 0 else psps
                    inps.append(pool.tile([P, D_H], F32,
                                          tag="inp" if j == 0 else "ps",
                                          name=f"inp_{b}"))
                    recs.append(psps.tile([P, D_H], F32, tag="ps",
                                          name=f"rec_{b}"))
                for c in range(KH):
                    for j in range(n):
                        nc.tensor.matmul(t0s[j], lhsT=hts[j][:, c, :],
                                         rhs=w1sb[KX + c], start=(c == 0),
                                         stop=False)
                    for j in range(n):
                        nc.tensor.matmul(recs[j], lhsT=hts[j][:, c, :],
                                         rhs=wasb[c], start=(c == 0),
                                         stop=(c == KH - 1))
                rec_ss = []
                for j, b in enumerate(bts):
                    rec_s = ep.tile([P, D_H], F32, tag="rec_s",
                                    name=f"recs_{b}")
                    nc.vector.tensor_copy(out=rec_s, in_=recs[j])
                    rec_ss.append(rec_s)
                for c in range(KX):
                    for j in range(n):
                        nc.tensor.matmul(t0s[j], lhsT=xts[j][:, c, :],
                                         rhs=w1sb[c], start=False,
                                         stop=(c == KX - 1))
                    for j in range(n):
                        nc.tensor.matmul(inps[j], lhsT=xts[j][:, c, :],
                                         rhs=wbsb[c], start=(c == 0),
                                         stop=(c == KX - 1))
                for j, b in enumerate(bts):
                    inp_s = ep.tile([P, D_H], F32, tag="inp_s",
                                    name=f"inps_{b}")
                    nc.vector.tensor_copy(out=inp_s, in_=inps[j])
                    state[b] = [t0s[j], inp_s, rec_ss[j], hns[j], None]

            def emit_mm1(bt, xt, ht, hn):
                t0 = pst0.tile([P, D_H], F32, tag="t0", name=f"t0_{bt}")
                inp = psinp.tile([P, D_H], F32, tag="inp", name=f"inp_{bt}")
                rec = psps.tile([P, D_H], F32, tag="ps", name=f"rec_{bt}")
                seqA = [(t0, xt[:, c, :], w1sb[c], c == 0, False)
                        for c in range(KX)]
                seqA += [(t0, ht[:, c, :], w1sb[KX + c], False, c == KH - 1)
                         for c in range(KH)]
                seqB = [(inp, xt[:, c, :], wbsb[c], c == 0, c == KX - 1)
                        for c in range(KX)]
                seqB += [(rec, ht[:, c, :], wasb[c], c == 0, c == KH - 1)
                         for c in range(KH)]
                # A0 A1 (B0 A2) (B1 A3) ... : banks alternate and inp's
                # first matmul trails the previous tile's PSUM eviction.
                order = seqA[:2]
                for i in range(len(seqB)):
                    order.append(seqB[i])
                    if i + 2 < len(seqA):
                        order.append(seqA[i + 2])
                for out_ps, lhsT, rhs, st, sp_ in order:
                    nc.tensor.matmul(out_ps, lhsT=lhsT, rhs=rhs,
                                     start=st, stop=sp_)

                # evict inp/rec to SBUF right away so their banks free for
                # the next tile's MM1 (inp bufs=1, rec shares the ps pool)
                inp_s = ep.tile([P, D_H], F32, tag="inp_s", name=f"inps_{bt}")
                nc.vector.tensor_copy(out=inp_s, in_=inp)
                rec_s = ep.tile([P, D_H], F32, tag="rec_s", name=f"recs_{bt}")
                nc.vector.tensor_copy(out=rec_s, in_=rec)
                state[bt] = [t0, inp_s, rec_s, hn, None]

            def emit_ln1(bt):
                t0, inp_s, rec_s, hn, _ = state[bt]
                if use("b1"):
                    nc.vector.tensor_add(t0, t0, vecs["b1"])
                st1 = stp.tile([P, 6], F32, tag="st1", name=f"st1_{bt}")
                nc.vector.bn_stats(st1, t0)
                mv1 = stp.tile([P, 2], F32, tag="mv1", name=f"mv1_{bt}")
                nc.vector.bn_aggr(mv1, st1)
                sd1 = stp.tile([P, 1], F32, tag="sd1", name=f"sd1_{bt}")
                nc.scalar.activation(sd1, mv1[:, 1:2], AF.Sqrt, bias=eps_t)
                rs1 = stp.tile([P, 1], F32, tag="rs1", name=f"rs1_{bt}")
                nc.vector.reciprocal(rs1, sd1)

                t2b = sp.tile([P, D_H], BF16, tag="t2b", name=f"t2b_{bt}")
                if use("g1") or use("be1"):
                    t1 = ep.tile([P, D_H], F32, tag="t1", name=f"t1_{bt}")
                    nc.vector.tensor_scalar(t1, t0, mv1[:, 0:1], rs1,
                                            ALU.subtract, ALU.mult)
                    if use("g1"):
                        nc.vector.tensor_mul(t1, t1, vecs["g1"])
                    if use("be1"):
                        nc.vector.tensor_add(t1, t1, vecs["be1"])
                    nc.scalar.activation(t2b, t1, AF.Gelu)
                else:
                    nmr = stp.tile([P, 1], F32, tag="nmr", name=f"nmr_{bt}")
                    nc.vector.tensor_scalar(nmr, mv1[:, 0:1], rs1, -1.0,
                                            ALU.mult, ALU.mult)
                    nc.scalar.activation(t2b, t0, AF.Gelu, bias=nmr, scale=rs1)
                state[bt][4] = t2b

            heads = {}

            def emit_head(bt, alt_params=False):
                t0, inp_s, rec_s, hn, t2b = state.pop(bt)

                t2T = sp.tile([P, KH, P], BF16, tag="t2T", name=f"t2T_{bt}")
                for f in range(KH):
                    tp = psps.tile([P, P], BF16, tag="ps", name=f"tp_{bt}_{f}")
                    nc.tensor.transpose(tp, t2b[:, f * P:(f + 1) * P], ident)
                    nc.vector.tensor_copy(out=t2T[:, f, :], in_=tp)

                if alt_params:
                    # the MM1 pools are idle once the last MM1 has been
                    # consumed - reuse their banks so this tail's MM2 does
                    # not wait for the previous tail's ACT chain to release
                    # the shared pool's slots
                    taus = pst0.tile([P, D_H], F32, tag="t0",
                                     name=f"taus_{bt}")
                    As = psps.tile([P, D_H], F32, tag="ps", name=f"As_{bt}")
                    Bs = psinp.tile([P, D_H], F32, tag="inp",
                                    name=f"Bs_{bt}")
                else:
                    taus = psps.tile([P, D_H], F32, tag="ps",
                                     name=f"taus_{bt}")
                    As = psps.tile([P, D_H], F32, tag="ps", name=f"As_{bt}")
                    Bs = psps.tile([P, D_H], F32, tag="ps", name=f"Bs_{bt}")
                for f in range(KH):
                    nc.tensor.matmul(taus, lhsT=t2T[:, f, :],
                                     rhs=w2sb[f][:, 0:D_H],
                                     start=(f == 0), stop=(f == KH - 1))
                    nc.tensor.matmul(As, lhsT=t2T[:, f, :],
                                     rhs=w2sb[f][:, D_H:2 * D_H],
                                     start=(f == 0), stop=(f == KH - 1))
                    nc.tensor.matmul(Bs, lhsT=t2T[:, f, :],
                                     rhs=w2sb[f][:, 2 * D_H:N2],
                                     start=(f == 0), stop=(f == KH - 1))
                if use("b2"):
                    nc.vector.tensor_add(taus, taus, vecs["b2"][:, 0:D_H])
                    nc.vector.tensor_add(As, As, vecs["b2"][:, D_H:2 * D_H])
                    nc.vector.tensor_add(Bs, Bs, vecs["b2"][:, 2 * D_H:N2])
                heads[bt] = (taus, As, Bs, inp_s, rec_s, hn)

            def emit_epi(bt):
                taus, As, Bs, inp_s, rec_s, hn = heads.pop(bt)
                sg = ep.tile([P, D_H], F32, tag="sg", name=f"sg_{bt}")
                tA = ep.tile([P, D_H], F32, tag="tA", name=f"tA_{bt}")
                tB = ep.tile([P, D_H], F32, tag="tB", name=f"tB_{bt}")
                st2 = stp.tile([P, 6], F32, tag="st2", name=f"st2_{bt}")

                # 0.1*h/(2*sigmoid(x)+1e-8) ~= 0.05*h*(1+exp(-x)): exact
                # sigmoid identity; avoids the slow DVE reciprocal
                nc.scalar.activation(sg, taus, AF.Exp, scale=-1.0)
                nc.vector.tensor_scalar(sg, sg, 1.0, 0.05, ALU.add, ALU.mult)
                if use("btau"):
                    nc.vector.tensor_mul(sg, sg, vecs["btau_inv"])
                nc.vector.tensor_mul(sg, hn, sg)
                nc.scalar.activation(tB, Bs, AF.Tanh)
                nc.vector.tensor_scalar(tB, tB, 0.5, 1.0, ALU.mult, ALU.add)
                nc.vector.tensor_mul(tB, inp_s, tB)        # inp * B_col
                nc.scalar.activation(tA, As, AF.Tanh)
                nc.vector.tensor_scalar(tA, tA, 0.5, 1.0, ALU.mult, ALU.add)
                nc.vector.tensor_mul(tA, rec_s, tA)        # rec * A_row
                nc.vector.tensor_add(tA, tA, tB)
                nc.scalar.activation(tA, tA, AF.Tanh)      # tanh(rec'+inp')
                nc.vector.tensor_scalar(tA, tA, 0.1, None, ALU.mult)
                nc.vector.tensor_sub(tA, tA, sg)           # 0.1*dhdt
                nc.vector.tensor_add(tA, hn, tA)           # h + 0.1*dhdt
                nc.vector.bn_stats(st2, tA)
                mv2 = stp.tile([P, 2], F32, tag="mv2", name=f"mv2_{bt}")
                nc.vector.bn_aggr(mv2, st2)
                sd2 = stp.tile([P, 1], F32, tag="sd2", name=f"sd2_{bt}")
                nc.scalar.activation(sd2, mv2[:, 1:2], AF.Sqrt, bias=eps_t)
                rs2 = stp.tile([P, 1], F32, tag="rs2", name=f"rs2_{bt}")
                nc.vector.reciprocal(rs2, sd2)

                y = ep.tile([P, D_H], F32, tag="y", name=f"y_{bt}")
                nc.vector.tensor_scalar(y, tA, mv2[:, 0:1], rs2,
                                        ALU.subtract, ALU.mult)
                if use("hng"):
                    nc.vector.tensor_mul(y, y, vecs["hng"])
                if use("hnb"):
                    nc.vector.tensor_add(y, y, vecs["hnb"])
                nc.sync.dma_start(out=out_e[bt * P:(bt + 1) * P, :], in_=y)

            def emit_tail(bt, alt_params=False):
                emit_head(bt, alt_params=alt_params)
                emit_epi(bt)

            def emit_mm1_p1(bt, xt, ht, hn):
                """First half of the last tile's MM1: only the t0
                accumulation, so gelu and both remaining heads can run
                while the inp/rec half still has PE work left."""
                t0 = pst0.tile([P, D_H], F32, tag="t0", name=f"t0_{bt}")
                inp = psinp.tile([P, D_H], F32, tag="inp", name=f"inp_{bt}")
                rec = psps.tile([P, D_H], F32, tag="ps", name=f"rec_{bt}")
                for c in range(KX):
                    nc.tensor.matmul(t0, lhsT=xt[:, c, :], rhs=w1sb[c],
                                     start=(c == 0), stop=False)
                for c in range(KH):
                    nc.tensor.matmul(t0, lhsT=ht[:, c, :], rhs=w1sb[KX + c],
                                     start=False, stop=(c == KH - 1))
                # epi reads inp/rec straight from PSUM for this tile
                state[bt] = [t0, inp, rec, hn, None]
                return inp, rec

            def emit_mm1_p2(bt, xt, ht, inp, rec):
                for c in range(KX):
                    nc.tensor.matmul(inp, lhsT=xt[:, c, :], rhs=wbsb[c],
                                     start=(c == 0), stop=(c == KX - 1))
                for c in range(KH):
                    nc.tensor.matmul(rec, lhsT=ht[:, c, :], rhs=wasb[c],
                                     start=(c == 0), stop=(c == KH - 1))

            emit_mm1_multi((0, 1), (xt0, xt1), (ht0, ht1), (hn0, hn1))
            emit_ln1(0)
            emit_ln1(1)
            for bt in range(2, NBT):
                xt = sp.tile([P, KX, P], BF16, tag="xt", name=f"xt_{bt}")
                nc.sync.dma_start(out=xt, in_=xT_e[bt])
                ht = sp.tile([P, KH, P], BF16, tag="ht", name=f"ht_{bt}")
                nc.sync.dma_start(out=ht, in_=hT_e[bt])
                hn = sp.tile([P, D_H], F32, tag="hn", name=f"hn_{bt}")
                nc.sync.dma_start(out=hn, in_=h_e[bt * P:(bt + 1) * P, :])
                if bt == NBT - 1:
                    # tail(5) moves ahead of MM1(7); MM1(7) itself is split:
                    # t0-half first, then gelu(7) + heads(6,7), then the
                    # inp/rec half, so both remaining epilogues overlap
                    # ~8us of PE work instead of trailing the last matmul
                    emit_head(bt - 2)
                    emit_epi(bt - 2)
                    inp7, rec7 = emit_mm1_p1(bt, xt, ht, hn)
                    emit_ln1(bt)
                    emit_head(bt - 1, alt_params=True)
                    emit_head(bt)
                    emit_mm1_p2(bt, xt, ht, inp7, rec7)
                    emit_epi(bt - 1)
                    emit_epi(bt)
                else:
                    emit_mm1(bt, xt, ht, hn)
                    emit_tail(bt - 2)
                    emit_ln1(bt)

    nc.compile()
    return nc


def _prep(x, h, W1, b1, ln1_g, ln1_b, W2, b2, base_tau, base_A, base_B,
          hn_g, hn_b):
    """Host-side shard prep. Returns (flags, in_maps)."""
    flags = set()
    checks = [("b1", b1, 0.0), ("g1", ln1_g, 1.0), ("be1", ln1_b, 0.0),
              ("b2", b2, 0.0), ("btau", base_tau, 1.0), ("hng", hn_g, 1.0),
              ("hnb", hn_b, 0.0)]
    vals = {"b1": b1, "g1": ln1_g, "be1": ln1_b, "b2": b2, "btau": base_tau,
            "hng": hn_g, "hnb": hn_b}
    for name, v, ident_val in checks:
        if not np.all(np.asarray(v) == ident_val):
            flags.add(name)

    w1 = np.ascontiguousarray(
        W1.reshape(KX + KH, P, D_H).transpose(1, 0, 2)).astype(_BF)
    wb = np.ascontiguousarray(
        base_B.reshape(KX, P, D_H).transpose(1, 0, 2)).astype(_BF)
    wa = np.ascontiguousarray(
        base_A.T.reshape(KH, P, D_H).transpose(1, 0, 2)).astype(_BF)
    w2 = np.ascontiguousarray(
        W2.reshape(KH, P, N2).transpose(1, 0, 2)).astype(_BF)

    in_maps = []
    for i in range(N_CORES):
        xl = x[i * B_LOC:(i + 1) * B_LOC]
        hl = h[i * B_LOC:(i + 1) * B_LOC]
        # xT[bt, k, c, b] = x[bt*128+b, c*128+k]
        xT = np.ascontiguousarray(
            xl.reshape(NBT, P, KX, P).transpose(0, 3, 2, 1)).astype(_BF)
        hT = np.ascontiguousarray(
            hl.reshape(NBT, P, KH, P).transpose(0, 3, 2, 1)).astype(_BF)
        m = {"xT": xT, "hT": hT, "h": np.ascontiguousarray(hl, np.float32),
             "w1": w1, "wb": wb, "wa": wa, "w2": w2}
        for name in flags:
            m[name] = np.ascontiguousarray(vals[name], np.float32)
        in_maps.append(m)
    return frozenset(flags), in_maps


def run(inputs: dict, trace: bool = False):
    """Compile (cached), execute on cores 0-7, gather. Returns (out, res)."""
    flags, in_maps = _prep(**inputs)
    nc = _graph_cache.get(flags)
    if nc is None:
        nc = _build(flags)
        _graph_cache[flags] = nc
    res = run_bass_kernel_spmd(nc, in_maps, core_ids=list(range(N_CORES)),
                               trace=trace)
    out = np.concatenate([res.results[i]["out"] for i in range(N_CORES)],
                         axis=0)
    return out, res


def kernel(**inputs) -> np.ndarray:
    out, _ = run(inputs, trace=False)
    return out


# revision 47
# speedup vs baseline: 1.0227x; 1.0140x over previous
"""AdaptiveCfCCell fused kernel for 8 TRN2 NeuronCores (pure data parallel).

Reference computation (per sample row):
    xh = [x, h]                                  # [B, 4608]
    t  = gelu(LN(xh @ W1 + b1) * g1 + be1)       # [B, 512]
    p  = t @ W2 + b2                             # [B, 1536]
    tau = base_tau * 2*sigmoid(p[:, :512])
    A_row = 1 + 0.5*tanh(p[:, 512:1024])
    B_col = 1 + 0.5*tanh(p[:, 1024:1536])
    rec = (h @ base_A.T) * A_row
    inp = (x @ base_B) * B_col
    dhdt = -h/(tau+1e-8) + tanh(rec + inp)
    out = LN(h + 0.1*dhdt) * hn_g + hn_b

Sharding: batch dim split 8 ways; weights replicated. Activations are kept
batch-major (batch rows on SBUF partitions); x and h are pre-transposed on the
host so each matmul lhsT tile ([K=128, M=128]) DMAs contiguously. All matmuls
run in bf16 with f32 PSUM accumulation; normalization/elementwise math is f32.
Affine vectors that are exactly identity (b=0 / g=1) are skipped at build time.
"""

import sys

sys.path.insert(0, "/opt/trn_rl_repo")

import numpy as np
import ml_dtypes

import concourse.bass as bass
import concourse.tile as tile
import concourse.mybir as mybir
from concourse import bacc
from concourse.bass_utils import run_bass_kernel_spmd
from concourse.masks import make_identity

AF = mybir.ActivationFunctionType
ALU = mybir.AluOpType
F32 = mybir.dt.float32
BF16 = mybir.dt.bfloat16

D_IN, D_H, BATCH = 4096, 512, 8192
N_CORES = 8
B_LOC = BATCH // N_CORES          # 1024 rows per core
P = 128                           # partitions
NBT = B_LOC // P                  # 8 batch tiles per core
KX = D_IN // P                    # 32 K-chunks over x features
KH = D_H // P                     # 4  K-chunks over h features
N2 = 3 * D_H                      # 1536

_BF = ml_dtypes.bfloat16

_graph_cache: dict[tuple, object] = {}


def _bcast(ap, p=P):
    """Partition-broadcast a 1-D DRAM row vector AP to [p, n]."""
    return bass.AP(tensor=ap.tensor, offset=ap.offset, ap=[[0, p], *ap.ap])


def _build(flags: frozenset):
    """Build + compile the SPMD graph. `flags` names the non-identity affine
    vectors that must actually be applied."""
    use = lambda k: k in flags

    nc = bacc.Bacc("TRN2", target_bir_lowering=False)

    xT_e = nc.dram_tensor("xT", [NBT, P, KX, P], BF16, kind="ExternalInput")
    hT_e = nc.dram_tensor("hT", [NBT, P, KH, P], BF16, kind="ExternalInput")
    h_e = nc.dram_tensor("h", [B_LOC, D_H], F32, kind="ExternalInput")
    w1_e = nc.dram_tensor("w1", [P, KX + KH, D_H], BF16, kind="ExternalInput")
    wb_e = nc.dram_tensor("wb", [P, KX, D_H], BF16, kind="ExternalInput")
    wa_e = nc.dram_tensor("wa", [P, KH, D_H], BF16, kind="ExternalInput")
    w2_e = nc.dram_tensor("w2", [P, KH, N2], BF16, kind="ExternalInput")
    out_e = nc.dram_tensor("out", [B_LOC, D_H], F32, kind="ExternalOutput")

    vec_e = {}
    for name, n in [("b1", D_H), ("g1", D_H), ("be1", D_H), ("b2", N2),
                    ("btau", D_H), ("hng", D_H), ("hnb", D_H)]:
        if use(name):
            vec_e[name] = nc.dram_tensor(name, [n], F32, kind="ExternalInput")

    with tile.TileContext(nc) as tc:
        with (
            tc.tile_pool(name="weights", bufs=1) as wp,
            tc.tile_pool(name="stream", bufs=3) as sp,
            tc.tile_pool(name="work", bufs=3) as ep,
            tc.tile_pool(name="stats", bufs=3) as stp,
            tc.tile_pool(name="pst0", bufs=2, space="PSUM") as pst0,
            tc.tile_pool(name="psinp", bufs=1, space="PSUM") as psinp,
            tc.tile_pool(name="psps", bufs=5, space="PSUM") as psps,
        ):
            # ---- resident weights -------------------------------------
            ident = wp.tile([P, P], BF16, tag="ident")
            make_identity(nc, ident)
            eps_t = wp.tile([P, 1], F32, tag="eps")
            nc.vector.memset(eps_t, 1e-5)

            # bt=0's activations and the weights stream on *different* DMA
            # engines (sync vs gpsimd), chunk-interleaved in exactly the
            # order MM1(0) consumes them, so the first matmul starts ~1-2us
            # in and never waits on the 11MB weight stream.
            # weights live in per-slab tiles (one DMA each); w1sb/wbsb/...
            # below are per-chunk [P, 512] views into them
            W1S = [(0, 2), (2, 6), (6, 12), (12, 20), (20, 28), (28, 32),
                   (32, 36)]
            WBS = [(0, 2), (2, 6), (6, 12), (12, 20), (20, 28), (28, 32)]
            w1slab = [wp.tile([P, s1 - s0, D_H], BF16, tag=f"w1s_{k}",
                              name=f"w1s_{k}") for k, (s0, s1) in enumerate(W1S)]
            wbslab = [wp.tile([P, s1 - s0, D_H], BF16, tag=f"wbs_{k}",
                              name=f"wbs_{k}") for k, (s0, s1) in enumerate(WBS)]
            waslab = wp.tile([P, KH, D_H], BF16, tag="was", name="was")
            w2slab = wp.tile([P, KH, N2], BF16, tag="w2s", name="w2s")

            def _chunk_views(slabs, bounds):
                views = []
                for k, (s0, s1) in enumerate(bounds):
                    for j in range(s1 - s0):
                        views.append(slabs[k][:, j, :])
                return views

            w1sb = _chunk_views(w1slab, W1S)
            wbsb = _chunk_views(wbslab, WBS)
            wasb = [waslab[:, c, :] for c in range(KH)]
            w2sb = [w2slab[:, f, :] for f in range(KH)]

            # Weights spread over four DMA issue queues (gpsimd/scalar/
            # vector/sync), each in MM1(0) consumption order, so bt0/bt1 are
            # not gated on a single ~200GB/s queue. xt0 goes first on sync
            # in four 256KB slabs so MM#0 starts ~1us after the preamble.
            xt0 = sp.tile([P, KX, P], BF16, tag="xt")
            ht0 = sp.tile([P, KH, P], BF16, tag="ht")
            hn0 = sp.tile([P, D_H], F32, tag="hn")
            xt1 = sp.tile([P, KX, P], BF16, tag="xt", name="xt_1")
            ht1 = sp.tile([P, KH, P], BF16, tag="ht", name="ht_1")
            hn1 = sp.tile([P, D_H], F32, tag="hn", name="hn_1")
            # h-part inputs lead their queues (the paired MM1 starts with
            # the h-part); sync then carries the xt0/xt1 slab stream
            nc.sync.dma_start(out=ht0, in_=hT_e[0])
            nc.sync.dma_start(out=ht1, in_=hT_e[1])
            for s0, s1 in [(0, 8), (8, 20), (20, 32)]:
                nc.sync.dma_start(out=xt0[:, s0:s1, :],
                                  in_=xT_e[0][:, s0:s1, :])
                nc.sync.dma_start(out=xt1[:, s0:s1, :],
                                  in_=xT_e[1][:, s0:s1, :])

            # slab DMAs are fully contiguous (weights are stored
            # partition-major in DRAM); slabs are spread over the three DMA
            # queues interleaved by the order MM1(0) consumes them
            # first h-weight chunk on its own small DMA so MM#0 starts
            # ~2us earlier; slabs balanced across the three queues roughly
            # by byte count in consumption order
            nc.gpsimd.dma_start(out=w1slab[len(W1S) - 1][:, 0:1, :],
                                in_=w1_e[:, KX:KX + 1, :])
            nc.scalar.dma_start(out=waslab[:, 0:1, :], in_=wa_e[:, 0:1, :])
            nc.gpsimd.dma_start(out=w1slab[len(W1S) - 1][:, 1:KH, :],
                                in_=w1_e[:, KX + 1:KX + KH, :])
            nc.scalar.dma_start(out=waslab[:, 1:KH, :], in_=wa_e[:, 1:KH, :])
            for k, (s0, s1) in enumerate(W1S[:-1]):
                nc.gpsimd.dma_start(out=w1slab[k], in_=w1_e[:, s0:s1, :])
            for k, (s0, s1) in enumerate(WBS):
                nc.scalar.dma_start(out=wbslab[k], in_=wb_e[:, s0:s1, :])
            nc.gpsimd.dma_start(out=hn0, in_=h_e[0:P, :])
            nc.scalar.dma_start(out=w2slab, in_=w2_e[:, :, :])
            nc.scalar.dma_start(out=hn1, in_=h_e[P:2 * P, :])

            vecs = {}
            for name, n in [("b1", D_H), ("g1", D_H), ("be1", D_H), ("b2", N2),
                            ("btau", D_H), ("hng", D_H), ("hnb", D_H)]:
                if use(name):
                    t = wp.tile([P, n], F32, tag=f"vec_{name}")
                    nc.gpsimd.dma_start(out=t, in_=_bcast(vec_e[name][:]))
                    vecs[name] = t
            if use("btau"):
                t = wp.tile([P, D_H], F32, tag="vec_btau_inv")
                nc.vector.reciprocal(t, vecs["btau"])
                vecs["btau_inv"] = t

            # ---- main loop over batch tiles ---------------------------
            # Software-pipelined: iteration bt emits MM1(bt) (t0/inp/rec
            # matmuls interleaved so consecutive matmuls hit alternating
            # PSUM banks -> ~216ns cadence), then transpose+MM2+epilogue of
            # bt-1 (their PE work lands after MM1(bt), hiding the LN1/gelu
            # latency), then LN1(bt). ACT order per iteration is
            # [Exp,Tanh x3,Sqrt | Sqrt,Gelu] -> 4 table loads per tile.
            state = {}

            def emit_mm1_multi(bts, xts, hts, hns):
                """MM1 for several batch tiles interleaved chunk-by-chunk:
                each weight chunk is consumed len(bts) times on arrival, so
                the PE stays busy while the 11MB weight stream is still in
                flight (the first tiles are DMA-bound, not PE-bound). The
                h-part runs first (it needs only ~1.1MB of inputs) and its
                rec accumulators are evicted immediately to free banks."""
                n = len(bts)
                t0s, inps, recs = [], [], []
                for j, b in enumerate(bts):
                    pool = pst0 if j < 2 else psps
                    t0s.append(pool.tile([P, D_H], F32,
                                         tag="t0" if j < 2 else "ps",
                                         name=f"t0_{b}"))
                    pool = psinp if j == 0 else psps
                    inps.append(pool.tile([P, D_H], F32,
                                          tag="inp" if j == 0 else "ps",
                                          name=f"inp_{b}"))
                    recs.append(psps.tile([P, D_H], F32, tag="ps",
                                          name=f"rec_{b}"))
                for c in range(KH):
                    for j in range(n):
                        nc.tensor.matmul(t0s[j], lhsT=hts[j][:, c, :],
                                         rhs=w1sb[KX + c], start=(c == 0),
                                         stop=False)
                    for j in range(n):
                        nc.tensor.matmul(recs[j], lhsT=hts[j][:, c, :],
                                         rhs=wasb[c], start=(c == 0),
                                         stop=(c == KH - 1))
                rec_ss = []
                for j, b in enumerate(bts):
                    rec_s = ep.tile([P, D_H], F32, tag="rec_s",
                                    name=f"recs_{b}")
                    nc.vector.tensor_copy(out=rec_s, in_=recs[j])
                    rec_ss.append(rec_s)
                for c in range(KX):
                    for j in range(n):
                        nc.tensor.matmul(t0s[j], lhsT=xts[j][:, c, :],
                                         rhs=w1sb[c], start=False,
                                         stop=(c == KX - 1))
                    for j in range(n):
                        nc.tensor.matmul(inps[j], lhsT=xts[j][:, c, :],
                                         rhs=wbsb[c], start=(c == 0),
                                         stop=(c == KX - 1))
                for j, b in enumerate(bts):
                    inp_s = ep.tile([P, D_H], F32, tag="inp_s",
                                    name=f"inps_{b}")
                    nc.vector.tensor_copy(out=inp_s, in_=inps[j])
                    state[b] = [t0s[j], inp_s, rec_ss[j], hns[j], None]

            def emit_mm1(bt, xt, ht, hn):
                t0 = pst0.tile([P, D_H], F32, tag="t0", name=f"t0_{bt}")
                inp = psinp.tile([P, D_H], F32, tag="inp", name=f"inp_{bt}")
                rec = psps.tile([P, D_H], F32, tag="ps", name=f"rec_{bt}")
                seqA = [(t0, xt[:, c, :], w1sb[c], c == 0, False)
                        for c in range(KX)]
                seqA += [(t0, ht[:, c, :], w1sb[KX + c], False, c == KH - 1)
                         for c in range(KH)]
                seqB = [(inp, xt[:, c, :], wbsb[c], c == 0, c == KX - 1)
                        for c in range(KX)]
                seqB += [(rec, ht[:, c, :], wasb[c], c == 0, c == KH - 1)
                         for c in range(KH)]
                # A0 A1 (B0 A2) (B1 A3) ... : banks alternate and inp's
                # first matmul trails the previous tile's PSUM eviction.
                order = seqA[:2]
                for i in range(len(seqB)):
                    order.append(seqB[i])
                    if i + 2 < len(seqA):
                        order.append(seqA[i + 2])
                for out_ps, lhsT, rhs, st, sp_ in order:
                    nc.tensor.matmul(out_ps, lhsT=lhsT, rhs=rhs,
                                     start=st, stop=sp_)

                # evict inp/rec to SBUF right away so their banks free for
                # the next tile's MM1 (inp bufs=1, rec shares the ps pool)
                inp_s = ep.tile([P, D_H], F32, tag="inp_s", name=f"inps_{bt}")
                nc.vector.tensor_copy(out=inp_s, in_=inp)
                rec_s = ep.tile([P, D_H], F32, tag="rec_s", name=f"recs_{bt}")
                nc.vector.tensor_copy(out=rec_s, in_=rec)
                state[bt] = [t0, inp_s, rec_s, hn, None]

            def emit_ln1(bt):
                t0, inp_s, rec_s, hn, _ = state[bt]
                if use("b1"):
                    nc.vector.tensor_add(t0, t0, vecs["b1"])
                st1 = stp.tile([P, 6], F32, tag="st1", name=f"st1_{bt}")
                nc.vector.bn_stats(st1, t0)
                mv1 = stp.tile([P, 2], F32, tag="mv1", name=f"mv1_{bt}")
                nc.vector.bn_aggr(mv1, st1)
                sd1 = stp.tile([P, 1], F32, tag="sd1", name=f"sd1_{bt}")
                nc.scalar.activation(sd1, mv1[:, 1:2], AF.Sqrt, bias=eps_t)
                rs1 = stp.tile([P, 1], F32, tag="rs1", name=f"rs1_{bt}")
                nc.vector.reciprocal(rs1, sd1)

                t2b = sp.tile([P, D_H], BF16, tag="t2b", name=f"t2b_{bt}")
                if use("g1") or use("be1"):
                    t1 = ep.tile([P, D_H], F32, tag="t1", name=f"t1_{bt}")
                    nc.vector.tensor_scalar(t1, t0, mv1[:, 0:1], rs1,
                                            ALU.subtract, ALU.mult)
                    if use("g1"):
                        nc.vector.tensor_mul(t1, t1, vecs["g1"])
                    if use("be1"):
                        nc.vector.tensor_add(t1, t1, vecs["be1"])
                    nc.scalar.activation(t2b, t1, AF.Gelu)
                else:
                    nmr = stp.tile([P, 1], F32, tag="nmr", name=f"nmr_{bt}")
                    nc.vector.tensor_scalar(nmr, mv1[:, 0:1], rs1, -1.0,
                                            ALU.mult, ALU.mult)
                    nc.scalar.activation(t2b, t0, AF.Gelu, bias=nmr, scale=rs1)
                state[bt][4] = t2b

            heads = {}

            def emit_head(bt, alt_params=False):
                t0, inp_s, rec_s, hn, t2b = state.pop(bt)

                t2T = sp.tile([P, KH, P], BF16, tag="t2T", name=f"t2T_{bt}")
                for f in range(KH):
                    tp = psps.tile([P, P], BF16, tag="ps", name=f"tp_{bt}_{f}")
                    nc.tensor.transpose(tp, t2b[:, f * P:(f + 1) * P], ident)
                    nc.vector.tensor_copy(out=t2T[:, f, :], in_=tp)

                if alt_params:
                    # the MM1 pools are idle once the last MM1 has been
                    # consumed - reuse their banks so this tail's MM2 does
                    # not wait for the previous tail's ACT chain to release
                    # the shared pool's slots
                    taus = pst0.tile([P, D_H], F32, tag="t0",
                                     name=f"taus_{bt}")
                    As = pst0.tile([P, D_H], F32, tag="t0", name=f"As_{bt}")
                    Bs = psinp.tile([P, D_H], F32, tag="inp",
                                    name=f"Bs_{bt}")
                else:
                    taus = psps.tile([P, D_H], F32, tag="ps",
                                     name=f"taus_{bt}")
                    As = psps.tile([P, D_H], F32, tag="ps", name=f"As_{bt}")
                    Bs = psps.tile([P, D_H], F32, tag="ps", name=f"Bs_{bt}")
                for f in range(KH):
                    nc.tensor.matmul(taus, lhsT=t2T[:, f, :],
                                     rhs=w2sb[f][:, 0:D_H],
                                     start=(f == 0), stop=(f == KH - 1))
                    nc.tensor.matmul(As, lhsT=t2T[:, f, :],
                                     rhs=w2sb[f][:, D_H:2 * D_H],
                                     start=(f == 0), stop=(f == KH - 1))
                    nc.tensor.matmul(Bs, lhsT=t2T[:, f, :],
                                     rhs=w2sb[f][:, 2 * D_H:N2],
                                     start=(f == 0), stop=(f == KH - 1))
                if use("b2"):
                    nc.vector.tensor_add(taus, taus, vecs["b2"][:, 0:D_H])
                    nc.vector.tensor_add(As, As, vecs["b2"][:, D_H:2 * D_H])
                    nc.vector.tensor_add(Bs, Bs, vecs["b2"][:, 2 * D_H:N2])
                heads[bt] = (taus, As, Bs, inp_s, rec_s, hn)

            def emit_epi(bt):
                taus, As, Bs, inp_s, rec_s, hn = heads.pop(bt)
                sg = ep.tile([P, D_H], F32, tag="sg", name=f"sg_{bt}")
                tA = ep.tile([P, D_H], F32, tag="tA", name=f"tA_{bt}")
                tB = ep.tile([P, D_H], F32, tag="tB", name=f"tB_{bt}")
                st2 = stp.tile([P, 6], F32, tag="st2", name=f"st2_{bt}")

                # 0.1*h/(2*sigmoid(x)+1e-8) ~= 0.05*h*(1+exp(-x)): exact
                # sigmoid identity; avoids the slow DVE reciprocal
                nc.scalar.activation(sg, taus, AF.Exp, scale=-1.0)
                nc.vector.tensor_scalar(sg, sg, 1.0, 0.05, ALU.add, ALU.mult)
                if use("btau"):
                    nc.vector.tensor_mul(sg, sg, vecs["btau_inv"])
                nc.vector.tensor_mul(sg, hn, sg)
                nc.scalar.activation(tA, As, AF.Tanh)
                nc.vector.tensor_scalar(tA, tA, 0.5, 1.0, ALU.mult, ALU.add)
                nc.vector.tensor_mul(tA, rec_s, tA)        # rec * A_row
                nc.scalar.activation(tB, Bs, AF.Tanh)
                nc.vector.tensor_scalar(tB, tB, 0.5, 1.0, ALU.mult, ALU.add)
                nc.vector.tensor_mul(tB, inp_s, tB)        # inp * B_col
                nc.vector.tensor_add(tA, tA, tB)
                nc.scalar.activation(tA, tA, AF.Tanh)      # tanh(rec'+inp')
                nc.vector.tensor_scalar(tA, tA, 0.1, None, ALU.mult)
                nc.vector.tensor_sub(tA, tA, sg)           # 0.1*dhdt
                nc.vector.tensor_add(tA, hn, tA)           # h + 0.1*dhdt
                nc.vector.bn_stats(st2, tA)
                mv2 = stp.tile([P, 2], F32, tag="mv2", name=f"mv2_{bt}")
                nc.vector.bn_aggr(mv2, st2)
                sd2 = stp.tile([P, 1], F32, tag="sd2", name=f"sd2_{bt}")
                nc.scalar.activation(sd2, mv2[:, 1:2], AF.Sqrt, bias=eps_t)
                rs2 = stp.tile([P, 1], F32, tag="rs2", name=f"rs2_{bt}")
                nc.vector.reciprocal(rs2, sd2)

                y = ep.tile([P, D_H], F32, tag="y", name=f"y_{bt}")
                nc.vector.tensor_scalar(y, tA, mv2[:, 0:1], rs2,
                                        ALU.subtract, ALU.mult)
                if use("hng"):
                    nc.vector.tensor_mul(y, y, vecs["hng"])
                if use("hnb"):
                    nc.vector.tensor_add(y, y, vecs["hnb"])
                nc.sync.dma_start(out=out_e[bt * P:(bt + 1) * P, :], in_=y)

            def emit_tail(bt, alt_params=False):
                emit_head(bt, alt_params=alt_params)
                emit_epi(bt)

            emit_mm1_multi((0, 1), (xt0, xt1), (ht0, ht1), (hn0, hn1))
            emit_ln1(0)
            emit_ln1(1)
            for bt in range(2, NBT):
                xt = sp.tile([P, KX, P], BF16, tag="xt", name=f"xt_{bt}")
                nc.sync.dma_start(out=xt, in_=xT_e[bt])
                ht = sp.tile([P, KH, P], BF16, tag="ht", name=f"ht_{bt}")
                nc.sync.dma_start(out=ht, in_=hT_e[bt])
                hn = sp.tile([P, D_H], F32, tag="hn", name=f"hn_{bt}")
                nc.sync.dma_start(out=hn, in_=h_e[bt * P:(bt + 1) * P, :])
                if bt == NBT - 1:
                    # tail(5) moves ahead of MM1(7): its matmuls run first
                    # (2.7us) and its epilogue then overlaps MM1(7) on
                    # DVE/ACT, leaving only two epilogues after the last MM
                    emit_head(bt - 2)
                    emit_epi(bt - 2)
                    emit_mm1(bt, xt, ht, hn)
                    emit_ln1(bt)
                    emit_head(bt - 1, alt_params=True)
                    emit_head(bt)
                    emit_epi(bt - 1)
                    emit_epi(bt)
                else:
                    emit_mm1(bt, xt, ht, hn)
                    emit_tail(bt - 2)
                    emit_ln1(bt)

    nc.compile()
    return nc


def _prep(x, h, W1, b1, ln1_g, ln1_b, W2, b2, base_tau, base_A, base_B,
          hn_g, hn_b):
    """Host-side shard prep. Returns (flags, in_maps)."""
    flags = set()
    checks = [("b1", b1, 0.0), ("g1", ln1_g, 1.0), ("be1", ln1_b, 0.0),
              ("b2", b2, 0.0), ("btau", base_tau, 1.0), ("hng", hn_g, 1.0),
              ("hnb", hn_b, 0.0)]
    vals = {"b1": b1, "g1": ln1_g, "be1": ln1_b, "b2": b2, "btau": base_tau,
            "hng": hn_g, "hnb": hn_b}
    for name, v, ident_val in checks:
        if not np.all(np.asarray(v) == ident_val):
            flags.add(name)

    w1 = np.ascontiguousarray(
        W1.reshape(KX + KH, P, D_H).transpose(1, 0, 2)).astype(_BF)
    wb = np.ascontiguousarray(
        base_B.reshape(KX, P, D_H).transpose(1, 0, 2)).astype(_BF)
    wa = np.ascontiguousarray(
        base_A.T.reshape(KH, P, D_H).transpose(1, 0, 2)).astype(_BF)
    w2 = np.ascontiguousarray(
        W2.reshape(KH, P, N2).transpose(1, 0, 2)).astype(_BF)

    in_maps = []
    for i in range(N_CORES):
        xl = x[i * B_LOC:(i + 1) * B_LOC]
        hl = h[i * B_LOC:(i + 1) * B_LOC]
        # xT[bt, k, c, b] = x[bt*128+b, c*128+k]
        xT = np.ascontiguousarray(
            xl.reshape(NBT, P, KX, P).transpose(0, 3, 2, 1)).astype(_BF)
        hT = np.ascontiguousarray(
            hl.reshape(NBT, P, KH, P).transpose(0, 3, 2, 1)).astype(_BF)
        m = {"xT": xT, "hT": hT, "h": np.ascontiguousarray(hl, np.float32),
             "w1": w1, "wb": wb, "wa": wa, "w2": w2}
        for name in flags:
            m[name] = np.ascontiguousarray(vals[name], np.float32)
        in_maps.append(m)
    return frozenset(flags), in_maps


def run(inputs: dict, trace: bool = False):
    """Compile (cached), execute on cores 0-7, gather. Returns (out, res)."""
    flags, in_maps = _prep(**inputs)
    nc = _graph_cache.get(flags)
    if nc is None:
        nc = _build(flags)
        _graph_cache[flags] = nc
    res = run_bass_kernel_spmd(nc, in_maps, core_ids=list(range(N_CORES)),
                               trace=trace)
    out = np.concatenate([res.results[i]["out"] for i in range(N_CORES)],
                         axis=0)
    return out, res


def kernel(**inputs) -> np.ndarray:
    out, _ = run(inputs, trace=False)
    return out
